# revision 33
# baseline (speedup 1.0000x reference)
"""Trainium2 Bass kernel for nn_Criterion_32830730011569.

Strategy: 8 cores = (image b in 0..3) x (H-half h in 0..1). The host
pre-gathers the matched channels (true_r = true[..., me], por_r = por[..., mq])
and ships the per-core pixel slices as fp8, so each core streams only
[18432, 96] x 2 plus the tiny occupancy tensors (~3.9MB vs 19MB of f32).

Dice per chunk of 36 pixel-rows: ACT computes exp(por_r); Z comes from a
pairwise-add tree (DVE 2x bf16 mode, vs always-1x tensor_reduce); the softmax
ep = exp * (1/Z) also runs 2x by replicating 1/Z into 4 contiguous bf16 lanes
so in1 has a unit-stride innermost dim. PE accumulates C[m_t, m_e] =
sum_pix true_r x softmax into one PSUM bank; trace(C) is the dice numerator
and sum(C) = sum(true_r) (softmax rows sum to 1) gives the denominator free.

The 7x7-window BCE gathers one contiguous 1159-pixel run per matched electron
from host-built channel-major [M, H*W] images (222KB vs 4.1MB), then extracts
the 7x7 with a strided copy. Occupancy CE streams fp8 logits plus the host
pre-gathered label logit. NLL/class stay f32 on pre-gathered small tensors.

Scheduling: ln/exp share one ACT table set, and every Ln is pinned (add_dep)
after the last dice exp so the set switches exactly once; gather-dependent
extraction and all tail DVE work are pinned after the last dice softmax so
they cannot head-of-line block the dice pipeline. Small inputs ride one
concatenated DMA on the gpsimd queue, keeping the sync queue free to dispatch
chunk DMAs immediately. Each core returns 8 partial sums; the host combines.
"""
import sys

sys.path.insert(0, "/opt/trn_rl_repo")
import numpy as np

B, H, W, Q, E, M, K, WIN = 4, 192, 192, 160, 96, 96, 4, 7
NO_E = 0.1
HALF = H // 2          # rows per core slice
NPIX = HALF * W        # 18432 pixels per slice
FULLPIX = H * W        # 36864 pixels per image
P = 128                # partitions
J = NPIX // P          # 144 pixels per partition (p-major)
CHUNKS = [(0, 36), (36, 36), (72, 36), (108, 36)]  # (start, size) per chunk
NCHUNK = len(CHUNKS)
RUN = (WIN - 1) * W + WIN  # 1159: contiguous window-row span
MAGIC = 8388608.0      # 2^23
NSM = 28               # used columns in the small-constant concat

_CACHE = {}


def _build_nc():
    import concourse.bass as bass
    import concourse.bacc as bacc
    import concourse.tile as tile
    from concourse.tile import add_dep_helper
    from concourse import mybir

    f32 = mybir.dt.float32
    i32 = mybir.dt.int32
    bf16 = mybir.dt.bfloat16
    f8 = mybir.dt.float8e3
    f8e4 = mybir.dt.float8e4
    AF = mybir.ActivationFunctionType
    OP = mybir.AluOpType
    AX = mybir.AxisListType

    nc = bacc.Bacc("TRN2", target_bir_lowering=False, debug=False, num_devices=8)

    # ---- external I/O ----
    por_sl = nc.dram_tensor("por_sl", [NPIX, M], f8, kind="ExternalInput")
    true_sl = nc.dram_tensor("true_sl", [NPIX, M], f8e4, kind="ExternalInput")
    occ_sl = nc.dram_tensor("occ_sl", [P, J, K], f8, kind="ExternalInput")
    xsel_d = nc.dram_tensor("xsel", [P, J], bf16, kind="ExternalInput")
    twin = nc.dram_tensor("twin", [M, FULLPIX], f8, kind="ExternalInput")
    bwin = nc.dram_tensor("bwin", [M, FULLPIX], f8, kind="ExternalInput")
    smalls_d = nc.dram_tensor("smalls", [P, NSM], f32, kind="ExternalInput")
    ident_d = nc.dram_tensor("ident", [M, M], f32, kind="ExternalInput")
    partials = nc.dram_tensor("partials", [1, 8], f32, kind="ExternalOutput")

    def bc(ap, pos, count):
        """Insert a stride-0 broadcast dim into an AP at free-dim position pos."""
        new = list(ap.ap)
        new.insert(pos, [0, count])
        return bass.AP(tensor=ap.tensor, offset=ap.offset, ap=new)

    def restride(ap, dims):
        """Replace the free dims of a 2D AP with explicit [step, count] pairs."""
        new_ap = [ap.ap[0]] + [list(d) for d in dims]
        return bass.AP(tensor=ap.tensor, offset=ap.offset, ap=new_ap)

    from contextlib import ExitStack

    with tile.TileContext(nc) as tc, ExitStack() as ctx:
        sing = ctx.enter_context(tc.tile_pool(name="sing", bufs=1))
        big = ctx.enter_context(tc.tile_pool(name="big", bufs=4))
        ps = ctx.enter_context(tc.tile_pool(name="ps", bufs=1, space="PSUM"))

        ones = sing.tile([P, 1], f32)
        nc.vector.memset(ones[:], 1.0)
        stats = sing.tile([P, 8], f32)
        nc.vector.memset(stats[:], 0.0)
        res = sing.tile([1, 8], f32)
        nc.vector.memset(res[:], 0.0)

        # one concatenated small-constant load on the gpsimd queue; the sync
        # queue stays free to dispatch the dice chunk DMAs immediately
        sm = sing.tile([P, NSM], f32)
        nc.gpsimd.dma_start(out=sm[:], in_=smalls_d.ap())
        ptsr = sm[0:M, 0:2]
        cenr = sm[0:M, 2:4]
        cholr = sm[0:M, 4:8]
        dr7 = sm[0:M, 8:15]
        wlo = sm[0:M, 15:16]
        whi = sm[0:M, 16:17]
        mbase = sm[0:M, 17:18]
        iel = sm[0:32, 18:23]
        lab = sm[0:32, 23:28]
        # ---------- window offsets (floor via 2^23 magic), feeds the gathers ----------
        rmag = sing.tile([M, 2], f32)
        nc.vector.tensor_scalar(out=rmag[:], in0=ptsr, scalar1=MAGIC, scalar2=-MAGIC,
                                op0=OP.add, op1=OP.add)
        gtm = sing.tile([M, 2], f32)
        nc.vector.tensor_tensor(out=gtm[:], in0=rmag[:], in1=ptsr, op=OP.is_gt)
        pixf = sing.tile([M, 2], f32)
        nc.vector.tensor_tensor(out=pixf[:], in0=rmag[:], in1=gtm[:], op=OP.subtract)
        # run start in full-image flat pixels: (r-3)*W + (c-3); always in
        # bounds because incidence points live in [4, 187]
        base = sing.tile([M, 1], f32)
        nc.vector.tensor_scalar(out=base[:], in0=pixf[:, 0:1], scalar1=float(W),
                                scalar2=float(-3 * W - 3), op0=OP.mult, op1=OP.add)
        nc.vector.tensor_tensor(out=base[:], in0=base[:], in1=pixf[:, 1:2], op=OP.add)
        soff = sing.tile([M, 1], f32)
        nc.vector.tensor_scalar(out=soff[:], in0=base[:], scalar1=mbase, scalar2=MAGIC,
                                op0=OP.add, op1=OP.add)
        soff_i = sing.tile([M, 1], i32)
        nc.vector.tensor_scalar(out=soff_i[:], in0=soff[:].bitcast(i32), scalar1=0x007FFFFF,
                                scalar2=None, op0=OP.bitwise_and)
        p0m3 = sing.tile([M, 1], f32)
        nc.vector.tensor_scalar(out=p0m3[:], in0=pixf[:, 0:1], scalar1=-3.0, scalar2=None,
                                op0=OP.add)

        # per-window-row validity: absolute row r-3+dr inside this core's half
        rows7 = sing.tile([M, WIN], f32)
        nc.gpsimd.tensor_scalar(out=rows7[:], in0=dr7, scalar1=p0m3[:], scalar2=None,
                                op0=OP.add)
        v1 = sing.tile([M, WIN], f32)
        nc.gpsimd.tensor_scalar(out=v1[:], in0=rows7[:], scalar1=wlo, scalar2=None, op0=OP.is_ge)
        v2 = sing.tile([M, WIN], f32)
        nc.gpsimd.tensor_scalar(out=v2[:], in0=rows7[:], scalar1=whi, scalar2=None, op0=OP.is_le)
        valid = sing.tile([M, WIN], f32)
        nc.gpsimd.tensor_tensor(out=valid[:], in0=v1[:], in1=v2[:], op=OP.mult)

        # ---------- window gathers: one 1159-element run per matched electron ----------
        twr = sing.tile([M, RUN], f8)
        bwr = sing.tile([M, RUN], f8)
        twin_flat = bass.AP(tensor=twin.ap().tensor, offset=0, ap=[[1, M * FULLPIX], [1, 1]])
        bwin_flat = bass.AP(tensor=bwin.ap().tensor, offset=0, ap=[[1, M * FULLPIX], [1, 1]])
        nc.gpsimd.indirect_dma_start(
            out=twr[:], out_offset=None, in_=twin_flat,
            in_offset=bass.IndirectOffsetOnAxis(ap=soff_i[:, 0:1], axis=0))
        nc.gpsimd.indirect_dma_start(
            out=bwr[:], out_offset=None, in_=bwin_flat,
            in_offset=bass.IndirectOffsetOnAxis(ap=soff_i[:, 0:1], axis=0))
        occ_t = sing.tile([P, J, K], f8)
        nc.sync.dma_start(out=occ_t[:], in_=occ_sl.ap())
        xsel_t = sing.tile([P, J], bf16)
        nc.sync.dma_start(out=xsel_t[:], in_=xsel_d.ap())
        ident = sing.tile([M, M], f32)
        nc.sync.dma_start(out=ident[:], in_=ident_d.ap())

        # ---------- dice streaming ----------
        por_v = por_sl.ap().rearrange("(p j) m -> p j m", p=P)
        true_v = true_sl.ap().rearrange("(p j) m -> p j m", p=P)
        C_ps = ps.tile([M, M], f32)

        def dice_chunk(c):
            # the sync queue spends ~7us in NRT preamble before its first
            # dispatch; chunk 0 rides the ACT queue (HWDGE), chunk 1 gpsimd
            dma_q = nc.scalar if c == 0 else (nc.gpsimd if c == 1 else nc.sync)
            j0, jc = CHUNKS[c]
            sl = slice(j0, j0 + jc)
            from contextlib import nullcontext
            prio = tc.high_priority() if c == 0 else nullcontext()
            with prio:
                por_t = big.tile([P, jc, M], f8, tag=f"por{jc}")
                dma_q.dma_start(out=por_t[:], in_=por_v[:, sl, :])
                t_t = big.tile([P, jc, M], f8e4, tag=f"t{jc}")
                dma_q.dma_start(out=t_t[:], in_=true_v[:, sl, :])
            exp_t = big.tile([P, jc, M], bf16, tag=f"exp{jc}")
            exp_i = nc.scalar.activation(out=exp_t[:], in_=por_t[:], func=AF.Exp)
            # Z via pairwise-tree adds (DVE 2x bf16 mode) + one small reduce;
            # a tensor_reduce over the full 96 would run 1x. Pool must NOT
            # take tree levels: its SBUF port is shared with the DVE and a
            # concurrent big Pool op makes 2-port DVE instructions ~14x slower.
            zt1 = big.tile([P, jc, 48], bf16, tag=f"zt1{jc}")
            nc.vector.tensor_tensor(out=zt1[:], in0=exp_t[:, :, 0:48],
                                    in1=exp_t[:, :, 48:96], op=OP.add)
            zt2 = big.tile([P, jc, 24], bf16, tag=f"zt2{jc}")
            nc.vector.tensor_tensor(out=zt2[:], in0=zt1[:, :, 0:24],
                                    in1=zt1[:, :, 24:48], op=OP.add)
            zt3 = big.tile([P, jc, 12], bf16, tag=f"zt3{jc}")
            nc.vector.tensor_tensor(out=zt3[:], in0=zt2[:, :, 0:12],
                                    in1=zt2[:, :, 12:24], op=OP.add)
            z_t = big.tile([P, jc], f32, tag=f"z{jc}")
            nc.vector.reduce_sum(out=z_t[:], in_=zt3[:], axis=AX.X)
            rz = big.tile([P, jc], f32, tag=f"rz{jc}")
            nc.vector.reciprocal(out=rz[:], in_=z_t[:])
            # replicate 1/Z into 4 contiguous bf16 lanes so the softmax
            # product below has a unit-stride 16-bit in1 -> DVE 2x mode
            rz4 = big.tile([P, jc, 4], bf16, tag=f"rz4{jc}")
            nc.vector.tensor_scalar(out=rz4[:], in0=bc(rz[:], 2, 4), scalar1=1.0,
                                    scalar2=None, op0=OP.mult)
            ep = big.tile([P, jc, M], bf16, tag=f"ep{jc}")
            HJ = jc // 2
            for half in range(2):
                hs = slice(half * HJ, (half + 1) * HJ)
                ep_i = nc.vector.tensor_tensor(
                    out=ep[:, hs].rearrange("p j (g i) -> p j g i", i=4),
                    in0=exp_t[:, hs].rearrange("p j (g i) -> p j g i", i=4),
                    in1=bc(rz4[:, hs], 2, M // 4), op=OP.mult)
                for j in range(half * HJ, (half + 1) * HJ):
                    nc.tensor.matmul(out=C_ps[:], lhsT=t_t[:, j, :], rhs=ep[:, j, :],
                                     start=(c == 0 and j == 0),
                                     stop=(c == NCHUNK - 1 and j == jc - 1))
            return exp_i, ep_i

        hands = [dice_chunk(c) for c in range(NCHUNK)]
        last_exp, last_ep = hands[-1]

        def pin(inst, anchor, reason="pin"):
            add_dep_helper(inst.ins, anchor.ins, reason=reason)
            return inst

        def after_dice(inst):
            return pin(inst, hands[0][1], "after first dice softmax")

        def after_exps(inst):
            return pin(inst, last_exp, "ln after exps")

        # ---------- exp-side of occ / class / windows (same ACT table set) ----------
        e4 = sing.tile([P, J, K], bf16)
        pin(nc.scalar.activation(out=e4[:], in_=occ_t[:], func=AF.Exp), hands[0][0],
            "occ exp after first dice exp")
        expc = sing.tile([32, 5], f32)
        pin(nc.scalar.activation(out=expc[:], in_=iel, func=AF.Exp), hands[0][0],
            "class exp after first dice exp")
        tv = sing.tile([M, WIN * WIN], f32)
        after_dice(nc.vector.tensor_copy(out=tv[:].rearrange("m (a b) -> m a b", a=WIN),
                                         in_=restride(twr[:], [[W, WIN], [1, WIN]])))
        lg = sing.tile([M, WIN * WIN], f32)
        after_dice(nc.vector.tensor_copy(out=lg[:].rearrange("m (a b) -> m a b", a=WIN),
                                         in_=restride(bwr[:], [[W, WIN], [1, WIN]])))
        expw = sing.tile([M, WIN * WIN], f32)
        pin(nc.scalar.activation(out=expw[:], in_=lg[:], func=AF.Exp), hands[2][0],
            "window exp before last dice exp")
        s42 = sing.tile([P, J, 2], bf16)
        after_dice(nc.vector.tensor_tensor(out=s42[:], in0=e4[:, :, 0:2],
                                           in1=e4[:, :, 2:4], op=OP.add))
        s4 = sing.tile([P, J], f32)
        nc.vector.tensor_tensor(out=s4[:], in0=s42[:, :, 0], in1=s42[:, :, 1], op=OP.add)

        # ---------- NLL prelude (f32 — the only term that needs precision) ----------
        d_ = sing.tile([M, 2], f32)
        pin(nc.vector.tensor_tensor(out=d_[:], in0=ptsr, in1=cenr, op=OP.subtract),
            hands[0][1], "nll after first dice softmax")
        r00 = sing.tile([M, 1], f32)
        nc.vector.reciprocal(out=r00[:], in_=cholr[:, 0:1])
        r11 = sing.tile([M, 1], f32)
        nc.vector.reciprocal(out=r11[:], in_=cholr[:, 3:4])
        z0 = sing.tile([M, 1], f32)
        nc.vector.tensor_tensor(out=z0[:], in0=d_[:, 0:1], in1=r00[:], op=OP.mult)
        t1 = sing.tile([M, 1], f32)
        nc.vector.tensor_tensor(out=t1[:], in0=cholr[:, 2:3], in1=z0[:], op=OP.mult)
        nc.vector.tensor_tensor(out=t1[:], in0=d_[:, 1:2], in1=t1[:], op=OP.subtract)
        z1 = sing.tile([M, 1], f32)
        nc.vector.tensor_tensor(out=z1[:], in0=t1[:], in1=r11[:], op=OP.mult)
        sq = sing.tile([M, 1], f32)
        nc.vector.tensor_tensor(out=sq[:], in0=z0[:], in1=z0[:], op=OP.mult)
        sq1 = sing.tile([M, 1], f32)
        nc.vector.tensor_tensor(out=sq1[:], in0=z1[:], in1=z1[:], op=OP.mult)
        nc.vector.tensor_tensor(out=sq[:], in0=sq[:], in1=sq1[:], op=OP.add)
        ldet = sing.tile([M, 1], f32)
        nc.vector.tensor_tensor(out=ldet[:], in0=cholr[:, 0:1], in1=cholr[:, 3:4], op=OP.mult)

        # ---------- Ln cluster (single ACT table switch, after all exps) ----------
        lse = sing.tile([P, J], f32)
        after_exps(nc.scalar.activation(out=lse[:], in_=s4[:], func=AF.Ln))
        sp = sing.tile([32, 5], f32)
        after_exps(nc.scalar.activation(out=sp[:], in_=expc[:], func=AF.Ln, bias=1.0))
        lnd = sing.tile([M, 1], f32)
        after_exps(nc.scalar.activation(out=lnd[:], in_=ldet[:], func=AF.Ln))
        spw = sing.tile([M, WIN * WIN], f32)
        after_exps(nc.scalar.activation(out=spw[:], in_=expw[:], func=AF.Ln, bias=1.0))

        # ---------- finishers ----------
        # occupancy CE
        d4 = sing.tile([P, J], f32)
        nc.gpsimd.tensor_tensor(out=d4[:], in0=lse[:], in1=xsel_t[:], op=OP.subtract)
        nc.vector.reduce_sum(out=stats[:, 4:5], in_=d4[:], axis=AX.X)
        # class loss (32 partitions, folded via the final ones-matmul)
        t9 = sing.tile([32, 5], f32)
        nc.vector.tensor_scalar(out=t9[:], in0=sp[:], scalar1=0.9, scalar2=None, op0=OP.mult)
        nc.vector.tensor_tensor(out=t9[:], in0=t9[:], in1=iel, op=OP.subtract)
        nc.vector.tensor_tensor(out=t9[:], in0=t9[:], in1=lab, op=OP.mult)
        nc.vector.reduce_sum(out=stats[0:32, 6:7], in_=t9[:], axis=AX.X)
        nc.vector.reduce_sum(out=stats[0:32, 5:6], in_=sp[:], axis=AX.X)
        # nll
        nc.vector.tensor_scalar(out=sq[:], in0=sq[:], scalar1=0.5,
                                scalar2=float(np.log(2.0 * np.pi)), op0=OP.mult, op1=OP.add)
        nc.vector.tensor_tensor(out=stats[0:M, 0:1], in0=sq[:], in1=lnd[:], op=OP.add)
        # window bce
        prw = sing.tile([M, WIN * WIN], f32)
        nc.gpsimd.tensor_tensor(out=prw[:], in0=lg[:], in1=tv[:], op=OP.mult)
        nc.gpsimd.tensor_tensor(out=prw[:], in0=spw[:], in1=prw[:], op=OP.subtract)
        valid49 = sing.tile([M, WIN * WIN], f32)
        nc.gpsimd.tensor_copy(out=valid49[:].rearrange("m (a b) -> m a b", a=WIN),
                              in_=bc(valid[:], 2, WIN))
        scr_w = sing.tile([M, WIN * WIN], f32)
        nc.gpsimd.tensor_tensor(out=scr_w[:], in0=prw[:], in1=valid49[:], op=OP.mult)
        nc.vector.reduce_sum(out=stats[0:M, 1:2], in_=scr_w[:], axis=AX.X)
        # dice: trace(C) and sum(C), read straight from PSUM
        scr_c = sing.tile([M, M], f32)
        nc.vector.tensor_tensor(out=scr_c[:], in0=C_ps[:], in1=ident[:], op=OP.mult)
        nc.vector.reduce_sum(out=stats[0:M, 2:3], in_=scr_c[:], axis=AX.X)
        nc.vector.reduce_sum(out=stats[0:M, 3:4], in_=C_ps[:], axis=AX.X)

        # ---------- final cross-partition reduction ----------
        fin_ps = ps.tile([1, 8], f32)
        nc.tensor.matmul(out=fin_ps[:], lhsT=ones[:], rhs=stats[:], start=True, stop=True)
        nc.vector.tensor_copy(out=res[:, 0:8], in_=fin_ps[:])
        nc.sync.dma_start(out=partials.ap(), in_=res[:])

    nc.compile()
    return nc


def _get_nc():
    if "nc" not in _CACHE:
        _CACHE["nc"] = _build_nc()
    return _CACHE["nc"]


def make_in_maps(is_electron_logit, true_segmap, binary_mask_logits, portion_logits,
                 incidence_points, positions, chol, occupancy_logits, occupancy_true,
                 matched_q, matched_e):
    import ml_dtypes
    f = np.float32
    f8 = ml_dtypes.float8_e3m4
    f8e4 = ml_dtypes.float8_e4m3
    bf = ml_dtypes.bfloat16
    ident = np.eye(M, dtype=f)
    in_maps = []
    for b in range(B):
        me = np.asarray(matched_e[b])
        mq = np.asarray(matched_q[b])
        true_r = np.asarray(true_segmap[b])[:, :, me]          # [H, W, M]
        por_r = np.asarray(portion_logits[b])[:, :, mq]        # [H, W, M]
        bin_r = np.asarray(binary_mask_logits[b])[:, :, mq]    # [H, W, M]
        twin_b = np.ascontiguousarray(true_r.reshape(FULLPIX, M).T).astype(f8)
        bwin_b = np.ascontiguousarray(bin_r.reshape(FULLPIX, M).T).astype(f8)
        iel = np.asarray(is_electron_logit, dtype=f).reshape(B, Q)[b].reshape(32, 5)
        lab = np.zeros(Q, dtype=f)
        lab[mq] = 1.0
        lab = lab.reshape(32, 5)
        occ_b = np.asarray(occupancy_logits[b], dtype=f)
        occt_b = np.asarray(occupancy_true[b])
        xsel_b = np.take_along_axis(occ_b.reshape(FULLPIX, K),
                                    occt_b.reshape(FULLPIX, 1), axis=1)
        for h in range(2):
            sl = slice(h * HALF, (h + 1) * HALF)
            psl = slice(h * NPIX, (h + 1) * NPIX)
            smalls = np.zeros((P, NSM), dtype=f)
            smalls[0:M, 0:2] = np.asarray(incidence_points[b], dtype=f)[me]
            smalls[0:M, 2:4] = np.asarray(positions[b], dtype=f)[mq]
            smalls[0:M, 4:8] = np.asarray(chol[b], dtype=f).reshape(Q, 4)[mq]
            smalls[0:M, 8:15] = np.arange(WIN, dtype=f)[None, :]
            smalls[0:M, 15] = float(h * HALF)
            smalls[0:M, 16] = float(h * HALF + HALF - 1)
            smalls[0:M, 17] = np.arange(M, dtype=f) * FULLPIX
            smalls[0:32, 18:23] = iel
            smalls[0:32, 23:28] = lab
            in_maps.append(dict(
                por_sl=np.ascontiguousarray(por_r[sl]).reshape(NPIX, M).astype(f8),
                true_sl=np.ascontiguousarray(true_r[sl]).reshape(NPIX, M).astype(f8e4),
                occ_sl=np.ascontiguousarray(occ_b[sl]).reshape(P, J, K).astype(f8),
                xsel=np.ascontiguousarray(xsel_b[psl]).reshape(P, J).astype(bf),
                twin=twin_b, bwin=bwin_b,
                smalls=smalls, ident=ident,
            ))
    return in_maps


def combine(partials_list):
    s = np.stack([np.asarray(p, dtype=np.float64).reshape(8) for p in partials_list])
    # slots: 0=nll_sum 1=bce_sum 2=trace(C) 3=sum(C)=sum_true 4=occ_sum
    # 5=softplus_sum 6=matched(0.9*sp - x) sum
    class_loss = (NO_E * s[0::2, 5].sum() + s[0::2, 6].sum()) / (B * Q)
    nll_loss = s[0::2, 0].sum() / (B * M)
    bce_loss = s[:, 1].sum() / (B * M * WIN * WIN)
    occ_loss = s[:, 4].sum() / (B * H * W)
    dice = 0.0
    for b in range(B):
        num = 2.0 * (s[2 * b, 2] + s[2 * b + 1, 2])
        den = s[2 * b, 3] + s[2 * b + 1, 3] + H * W
        dice += 1.0 - (num + 1.0) / (den + 1.0)
    dice_loss = dice / B
    return np.float32(class_loss + bce_loss + dice_loss + nll_loss + occ_loss)


def kernel(**inputs):
    from concourse.bass_utils import run_bass_kernel_spmd
    nc = _get_nc()
    in_maps = make_in_maps(**{k: np.asarray(v) for k, v in inputs.items()})
    r = run_bass_kernel_spmd(nc, in_maps, list(range(8)))
    return combine([r.results[c]["partials"] for c in range(8)])


# revision 35
# speedup vs baseline: 1.0070x; 1.0070x over previous
"""Trainium2 Bass kernel for nn_Criterion_32830730011569.

Strategy: 8 cores = (image b in 0..3) x (H-half h in 0..1). The host
pre-gathers the matched channels (true_r = true[..., me], por_r = por[..., mq])
and ships the per-core pixel slices as fp8, so each core streams only
[18432, 96] x 2 plus the tiny occupancy tensors (~3.9MB vs 19MB of f32).

Dice per chunk of 36 pixel-rows per partition: ACT computes exp(por_r); Z
comes from a pairwise-add tree + small reduce (DVE 2x bf16 mode, vs the
always-1x full tensor_reduce); the softmax ep = exp * (1/Z) also runs 2x by
replicating 1/Z into 4 contiguous bf16 lanes so in1 has a unit-stride 16-bit
innermost dim. PE accumulates C[m_t, m_e] = sum_pix true_r x softmax into one
PSUM bank (fp8e4 stationary, two per-chunk softmax halves so PE starts
mid-chunk); trace(C) is the dice numerator and sum(C) = sum(true_r) (softmax
rows sum to 1) gives the denominator for free.

The 7x7-window BCE gathers one contiguous 1159-pixel run per matched electron
from host-built channel-major [M, H*W] images (222KB vs 4.1MB), then extracts
the 7x7 with a strided copy. Occupancy CE streams fp8 logits plus the host
pre-gathered label logit. NLL/class stay f32 on pre-gathered small tensors.

Scheduling: ln/exp share one ACT table set, and every Ln is pinned (add_dep)
after the last dice exp so the set switches exactly once; gather-dependent
extraction and all tail DVE work are pinned after the last dice softmax so
they cannot head-of-line block the dice pipeline. Small inputs ride one
concatenated DMA on the gpsimd queue, keeping the sync queue free to dispatch
chunk DMAs immediately. Each core returns 8 partial sums; the host combines.
"""
import sys

sys.path.insert(0, "/opt/trn_rl_repo")
import numpy as np

B, H, W, Q, E, M, K, WIN = 4, 192, 192, 160, 96, 96, 4, 7
NO_E = 0.1
HALF = H // 2          # rows per core slice
NPIX = HALF * W        # 18432 pixels per slice
FULLPIX = H * W        # 36864 pixels per image
P = 128                # partitions
J = NPIX // P          # 144 pixels per partition (p-major)
CHUNKS = [(0, 36), (36, 36), (72, 36), (108, 36)]  # (start, size) per chunk
NCHUNK = len(CHUNKS)
RUN = (WIN - 1) * W + WIN  # 1159: contiguous window-row span
MAGIC = 8388608.0      # 2^23
NSM = 28               # used columns in the small-constant concat

_CACHE = {}


def _build_nc():
    import concourse.bass as bass
    import concourse.bacc as bacc
    import concourse.tile as tile
    from concourse.tile import add_dep_helper
    from concourse import mybir

    f32 = mybir.dt.float32
    i32 = mybir.dt.int32
    bf16 = mybir.dt.bfloat16
    f8 = mybir.dt.float8e3
    f8e4 = mybir.dt.float8e4
    AF = mybir.ActivationFunctionType
    OP = mybir.AluOpType
    AX = mybir.AxisListType

    nc = bacc.Bacc("TRN2", target_bir_lowering=False, debug=False, num_devices=8)

    # ---- external I/O ----
    por_sl = nc.dram_tensor("por_sl", [NPIX, M], f8, kind="ExternalInput")
    true_sl = nc.dram_tensor("true_sl", [NPIX, M], f8e4, kind="ExternalInput")
    occ_sl = nc.dram_tensor("occ_sl", [P, J, K], f8, kind="ExternalInput")
    xsel_d = nc.dram_tensor("xsel", [P, J], bf16, kind="ExternalInput")
    twin = nc.dram_tensor("twin", [M, FULLPIX], f8, kind="ExternalInput")
    bwin = nc.dram_tensor("bwin", [M, FULLPIX], f8, kind="ExternalInput")
    smalls_d = nc.dram_tensor("smalls", [P, NSM], f32, kind="ExternalInput")
    ident_d = nc.dram_tensor("ident", [M, M], f32, kind="ExternalInput")
    partials = nc.dram_tensor("partials", [1, 8], f32, kind="ExternalOutput")

    def bc(ap, pos, count):
        """Insert a stride-0 broadcast dim into an AP at free-dim position pos."""
        new = list(ap.ap)
        new.insert(pos, [0, count])
        return bass.AP(tensor=ap.tensor, offset=ap.offset, ap=new)

    def restride(ap, dims):
        """Replace the free dims of a 2D AP with explicit [step, count] pairs."""
        new_ap = [ap.ap[0]] + [list(d) for d in dims]
        return bass.AP(tensor=ap.tensor, offset=ap.offset, ap=new_ap)

    from contextlib import ExitStack

    with tile.TileContext(nc) as tc, ExitStack() as ctx:
        sing = ctx.enter_context(tc.tile_pool(name="sing", bufs=1))
        big = ctx.enter_context(tc.tile_pool(name="big", bufs=3))
        ps = ctx.enter_context(tc.tile_pool(name="ps", bufs=1, space="PSUM"))

        ones = sing.tile([P, 1], f32)
        nc.vector.memset(ones[:], 1.0)
        stats = sing.tile([P, 8], f32)
        nc.vector.memset(stats[:], 0.0)
        res = sing.tile([1, 8], f32)
        nc.vector.memset(res[:], 0.0)

        # one concatenated small-constant load on the gpsimd queue; the sync
        # queue stays free to dispatch the dice chunk DMAs immediately
        sm = sing.tile([P, NSM], f32)
        nc.gpsimd.dma_start(out=sm[:], in_=smalls_d.ap())
        ptsr = sm[0:M, 0:2]
        cenr = sm[0:M, 2:4]
        cholr = sm[0:M, 4:8]
        dr7 = sm[0:M, 8:15]
        wlo = sm[0:M, 15:16]
        whi = sm[0:M, 16:17]
        mbase = sm[0:M, 17:18]
        iel = sm[0:32, 18:23]
        lab = sm[0:32, 23:28]
        # ---------- window offsets (floor via 2^23 magic), feeds the gathers ----------
        rmag = sing.tile([M, 2], f32)
        nc.vector.tensor_scalar(out=rmag[:], in0=ptsr, scalar1=MAGIC, scalar2=-MAGIC,
                                op0=OP.add, op1=OP.add)
        gtm = sing.tile([M, 2], f32)
        nc.vector.tensor_tensor(out=gtm[:], in0=rmag[:], in1=ptsr, op=OP.is_gt)
        pixf = sing.tile([M, 2], f32)
        nc.vector.tensor_tensor(out=pixf[:], in0=rmag[:], in1=gtm[:], op=OP.subtract)
        # run start in full-image flat pixels: (r-3)*W + (c-3); always in
        # bounds because incidence points live in [4, 187]
        base = sing.tile([M, 1], f32)
        nc.vector.tensor_scalar(out=base[:], in0=pixf[:, 0:1], scalar1=float(W),
                                scalar2=float(-3 * W - 3), op0=OP.mult, op1=OP.add)
        nc.vector.tensor_tensor(out=base[:], in0=base[:], in1=pixf[:, 1:2], op=OP.add)
        soff = sing.tile([M, 1], f32)
        nc.vector.tensor_scalar(out=soff[:], in0=base[:], scalar1=mbase, scalar2=MAGIC,
                                op0=OP.add, op1=OP.add)
        soff_i = sing.tile([M, 1], i32)
        nc.vector.tensor_scalar(out=soff_i[:], in0=soff[:].bitcast(i32), scalar1=0x007FFFFF,
                                scalar2=None, op0=OP.bitwise_and)
        p0m3 = sing.tile([M, 1], f32)
        nc.vector.tensor_scalar(out=p0m3[:], in0=pixf[:, 0:1], scalar1=-3.0, scalar2=None,
                                op0=OP.add)

        # per-window-row validity: absolute row r-3+dr inside this core's half
        rows7 = sing.tile([M, WIN], f32)
        nc.gpsimd.tensor_scalar(out=rows7[:], in0=dr7, scalar1=p0m3[:], scalar2=None,
                                op0=OP.add)
        v1 = sing.tile([M, WIN], f32)
        nc.gpsimd.tensor_scalar(out=v1[:], in0=rows7[:], scalar1=wlo, scalar2=None, op0=OP.is_ge)
        v2 = sing.tile([M, WIN], f32)
        nc.gpsimd.tensor_scalar(out=v2[:], in0=rows7[:], scalar1=whi, scalar2=None, op0=OP.is_le)
        valid = sing.tile([M, WIN], f32)
        nc.gpsimd.tensor_tensor(out=valid[:], in0=v1[:], in1=v2[:], op=OP.mult)

        # ---------- window gathers: one 1159-element run per matched electron ----------
        twr = sing.tile([M, RUN], f8)
        bwr = sing.tile([M, RUN], f8)
        twin_flat = bass.AP(tensor=twin.ap().tensor, offset=0, ap=[[1, M * FULLPIX], [1, 1]])
        bwin_flat = bass.AP(tensor=bwin.ap().tensor, offset=0, ap=[[1, M * FULLPIX], [1, 1]])
        nc.gpsimd.indirect_dma_start(
            out=twr[:], out_offset=None, in_=twin_flat,
            in_offset=bass.IndirectOffsetOnAxis(ap=soff_i[:, 0:1], axis=0))
        nc.gpsimd.indirect_dma_start(
            out=bwr[:], out_offset=None, in_=bwin_flat,
            in_offset=bass.IndirectOffsetOnAxis(ap=soff_i[:, 0:1], axis=0))
        occ_t = sing.tile([P, J, K], f8)
        nc.sync.dma_start(out=occ_t[:], in_=occ_sl.ap())
        xsel_t = sing.tile([P, J], bf16)
        nc.sync.dma_start(out=xsel_t[:], in_=xsel_d.ap())
        ident = sing.tile([M, M], f32)
        nc.sync.dma_start(out=ident[:], in_=ident_d.ap())

        # ---------- dice streaming ----------
        por_v = por_sl.ap().rearrange("(p j) m -> p j m", p=P)
        true_v = true_sl.ap().rearrange("(p j) m -> p j m", p=P)
        C_ps = ps.tile([M, M], f32)

        def dice_chunk(c):
            # the sync queue spends ~7us in NRT preamble before its first
            # dispatch; chunk 0 rides the ACT queue (HWDGE), chunk 1 gpsimd
            dma_q = nc.scalar if c == 0 else (nc.gpsimd if c == 1 else nc.sync)
            j0, jc = CHUNKS[c]
            sl = slice(j0, j0 + jc)
            from contextlib import nullcontext
            prio = tc.high_priority() if c == 0 else nullcontext()
            with prio:
                por_t = big.tile([P, jc, M], f8, tag=f"por{jc}")
                dma_q.dma_start(out=por_t[:], in_=por_v[:, sl, :])
                t_t = big.tile([P, jc, M], f8e4, tag=f"t{jc}")
                dma_q.dma_start(out=t_t[:], in_=true_v[:, sl, :])
            exp_t = big.tile([P, jc, M], bf16, tag=f"exp{jc}")
            exp_i = nc.scalar.activation(out=exp_t[:], in_=por_t[:], func=AF.Exp)
            # Z via pairwise-tree adds (DVE 2x bf16 mode) + one small reduce;
            # a tensor_reduce over the full 96 would run 1x. Pool must NOT
            # take tree levels: its SBUF port is shared with the DVE and a
            # concurrent big Pool op makes 2-port DVE instructions ~14x slower.
            zt1 = big.tile([P, jc, 48], bf16, tag=f"zt1{jc}")
            nc.vector.tensor_tensor(out=zt1[:], in0=exp_t[:, :, 0:48],
                                    in1=exp_t[:, :, 48:96], op=OP.add)
            zt2 = big.tile([P, jc, 24], bf16, tag=f"zt2{jc}")
            nc.vector.tensor_tensor(out=zt2[:], in0=zt1[:, :, 0:24],
                                    in1=zt1[:, :, 24:48], op=OP.add)
            zt3 = big.tile([P, jc, 12], bf16, tag=f"zt3{jc}")
            nc.vector.tensor_tensor(out=zt3[:], in0=zt2[:, :, 0:12],
                                    in1=zt2[:, :, 12:24], op=OP.add)
            z_t = big.tile([P, jc], f32, tag=f"z{jc}")
            nc.vector.reduce_sum(out=z_t[:], in_=zt3[:], axis=AX.X)
            rz = big.tile([P, jc], f32, tag=f"rz{jc}")
            nc.vector.reciprocal(out=rz[:], in_=z_t[:])
            # replicate 1/Z into 4 contiguous bf16 lanes so the softmax
            # product below has a unit-stride 16-bit in1 -> DVE 2x mode
            rz4 = big.tile([P, jc, 4], bf16, tag=f"rz4{jc}")
            nc.vector.tensor_scalar(out=rz4[:], in0=bc(rz[:], 2, 4), scalar1=1.0,
                                    scalar2=None, op0=OP.mult)
            ep = big.tile([P, jc, M], bf16, tag=f"ep{jc}")
            HJ = jc // 2
            for half in range(2):
                hs = slice(half * HJ, (half + 1) * HJ)
                ep_i = nc.vector.tensor_tensor(
                    out=ep[:, hs].rearrange("p j (g i) -> p j g i", i=4),
                    in0=exp_t[:, hs].rearrange("p j (g i) -> p j g i", i=4),
                    in1=bc(rz4[:, hs], 2, M // 4), op=OP.mult)
                for j in range(half * HJ, (half + 1) * HJ):
                    nc.tensor.matmul(out=C_ps[:], lhsT=t_t[:, j, :], rhs=ep[:, j, :],
                                     start=(c == 0 and j == 0),
                                     stop=(c == NCHUNK - 1 and j == jc - 1))
            return exp_i, ep_i

        hands = [dice_chunk(c) for c in range(NCHUNK)]
        last_exp, last_ep = hands[-1]

        def pin(inst, anchor, reason="pin"):
            add_dep_helper(inst.ins, anchor.ins, reason=reason)
            return inst

        def after_dice(inst):
            return pin(inst, hands[0][1], "after first dice softmax")

        def after_exps(inst):
            return pin(inst, last_exp, "ln after exps")

        # ---------- exp-side of occ / class / windows (same ACT table set) ----------
        e4 = sing.tile([P, J, K], bf16)
        pin(nc.scalar.activation(out=e4[:], in_=occ_t[:], func=AF.Exp), hands[0][0],
            "occ exp after first dice exp")
        expc = sing.tile([32, 5], f32)
        pin(nc.scalar.activation(out=expc[:], in_=iel, func=AF.Exp), hands[0][0],
            "class exp after first dice exp")
        tv = sing.tile([M, WIN * WIN], f32)
        after_dice(nc.vector.tensor_copy(out=tv[:].rearrange("m (a b) -> m a b", a=WIN),
                                         in_=restride(twr[:], [[W, WIN], [1, WIN]])))
        lg = sing.tile([M, WIN * WIN], f32)
        after_dice(nc.vector.tensor_copy(out=lg[:].rearrange("m (a b) -> m a b", a=WIN),
                                         in_=restride(bwr[:], [[W, WIN], [1, WIN]])))
        expw = sing.tile([M, WIN * WIN], f32)
        pin(nc.scalar.activation(out=expw[:], in_=lg[:], func=AF.Exp), hands[2][0],
            "window exp before last dice exp")
        s42 = sing.tile([P, J, 2], bf16)
        after_dice(nc.vector.tensor_tensor(out=s42[:], in0=e4[:, :, 0:2],
                                           in1=e4[:, :, 2:4], op=OP.add))
        s4 = sing.tile([P, J], f32)
        nc.vector.tensor_tensor(out=s4[:], in0=s42[:, :, 0], in1=s42[:, :, 1], op=OP.add)

        # ---------- NLL prelude (f32 — the only term that needs precision) ----------
        d_ = sing.tile([M, 2], f32)
        pin(nc.vector.tensor_tensor(out=d_[:], in0=ptsr, in1=cenr, op=OP.subtract),
            hands[0][1], "nll after first dice softmax")
        r00 = sing.tile([M, 1], f32)
        nc.vector.reciprocal(out=r00[:], in_=cholr[:, 0:1])
        r11 = sing.tile([M, 1], f32)
        nc.vector.reciprocal(out=r11[:], in_=cholr[:, 3:4])
        z0 = sing.tile([M, 1], f32)
        nc.vector.tensor_tensor(out=z0[:], in0=d_[:, 0:1], in1=r00[:], op=OP.mult)
        t1 = sing.tile([M, 1], f32)
        nc.vector.tensor_tensor(out=t1[:], in0=cholr[:, 2:3], in1=z0[:], op=OP.mult)
        nc.vector.tensor_tensor(out=t1[:], in0=d_[:, 1:2], in1=t1[:], op=OP.subtract)
        z1 = sing.tile([M, 1], f32)
        nc.vector.tensor_tensor(out=z1[:], in0=t1[:], in1=r11[:], op=OP.mult)
        sq = sing.tile([M, 1], f32)
        nc.vector.tensor_tensor(out=sq[:], in0=z0[:], in1=z0[:], op=OP.mult)
        sq1 = sing.tile([M, 1], f32)
        nc.vector.tensor_tensor(out=sq1[:], in0=z1[:], in1=z1[:], op=OP.mult)
        nc.vector.tensor_tensor(out=sq[:], in0=sq[:], in1=sq1[:], op=OP.add)
        ldet = sing.tile([M, 1], f32)
        nc.vector.tensor_tensor(out=ldet[:], in0=cholr[:, 0:1], in1=cholr[:, 3:4], op=OP.mult)

        # ---------- Ln cluster (single ACT table switch, after all exps) ----------
        lse = sing.tile([P, J], f32)
        after_exps(nc.scalar.activation(out=lse[:], in_=s4[:], func=AF.Ln))
        sp = sing.tile([32, 5], f32)
        after_exps(nc.scalar.activation(out=sp[:], in_=expc[:], func=AF.Ln, bias=1.0))
        lnd = sing.tile([M, 1], f32)
        after_exps(nc.scalar.activation(out=lnd[:], in_=ldet[:], func=AF.Ln))
        spw = sing.tile([M, WIN * WIN], f32)
        after_exps(nc.scalar.activation(out=spw[:], in_=expw[:], func=AF.Ln, bias=1.0))

        # ---------- finishers ----------
        # occupancy CE
        d4 = sing.tile([P, J], f32)
        nc.gpsimd.tensor_tensor(out=d4[:], in0=lse[:], in1=xsel_t[:], op=OP.subtract)
        nc.vector.reduce_sum(out=stats[:, 4:5], in_=d4[:], axis=AX.X)
        # class loss (32 partitions, folded via the final ones-matmul)
        t9 = sing.tile([32, 5], f32)
        nc.vector.tensor_scalar(out=t9[:], in0=sp[:], scalar1=0.9, scalar2=None, op0=OP.mult)
        nc.vector.tensor_tensor(out=t9[:], in0=t9[:], in1=iel, op=OP.subtract)
        nc.vector.tensor_tensor(out=t9[:], in0=t9[:], in1=lab, op=OP.mult)
        nc.vector.reduce_sum(out=stats[0:32, 6:7], in_=t9[:], axis=AX.X)
        nc.vector.reduce_sum(out=stats[0:32, 5:6], in_=sp[:], axis=AX.X)
        # nll
        nc.vector.tensor_scalar(out=sq[:], in0=sq[:], scalar1=0.5,
                                scalar2=float(np.log(2.0 * np.pi)), op0=OP.mult, op1=OP.add)
        nc.vector.tensor_tensor(out=stats[0:M, 0:1], in0=sq[:], in1=lnd[:], op=OP.add)
        # window bce
        prw = sing.tile([M, WIN * WIN], f32)
        nc.gpsimd.tensor_tensor(out=prw[:], in0=lg[:], in1=tv[:], op=OP.mult)
        nc.gpsimd.tensor_tensor(out=prw[:], in0=spw[:], in1=prw[:], op=OP.subtract)
        valid49 = sing.tile([M, WIN * WIN], f32)
        nc.gpsimd.tensor_copy(out=valid49[:].rearrange("m (a b) -> m a b", a=WIN),
                              in_=bc(valid[:], 2, WIN))
        scr_w = sing.tile([M, WIN * WIN], f32)
        nc.gpsimd.tensor_tensor(out=scr_w[:], in0=prw[:], in1=valid49[:], op=OP.mult)
        nc.vector.reduce_sum(out=stats[0:M, 1:2], in_=scr_w[:], axis=AX.X)
        # dice: trace(C) and sum(C), read straight from PSUM
        scr_c = sing.tile([M, M], f32)
        nc.vector.tensor_tensor(out=scr_c[:], in0=C_ps[:], in1=ident[:], op=OP.mult)
        nc.vector.reduce_sum(out=stats[0:M, 2:3], in_=scr_c[:], axis=AX.X)
        nc.vector.reduce_sum(out=stats[0:M, 3:4], in_=C_ps[:], axis=AX.X)

        # ---------- final cross-partition reduction ----------
        fin_ps = ps.tile([1, 8], f32)
        nc.tensor.matmul(out=fin_ps[:], lhsT=ones[:], rhs=stats[:], start=True, stop=True)
        nc.vector.tensor_copy(out=res[:, 0:8], in_=fin_ps[:])
        nc.sync.dma_start(out=partials.ap(), in_=res[:])

    nc.compile()
    return nc


def _get_nc():
    if "nc" not in _CACHE:
        _CACHE["nc"] = _build_nc()
    return _CACHE["nc"]


def make_in_maps(is_electron_logit, true_segmap, binary_mask_logits, portion_logits,
                 incidence_points, positions, chol, occupancy_logits, occupancy_true,
                 matched_q, matched_e):
    import ml_dtypes
    f = np.float32
    f8 = ml_dtypes.float8_e3m4
    f8e4 = ml_dtypes.float8_e4m3
    bf = ml_dtypes.bfloat16
    ident = np.eye(M, dtype=f)
    in_maps = []
    for b in range(B):
        me = np.asarray(matched_e[b])
        mq = np.asarray(matched_q[b])
        true_r = np.asarray(true_segmap[b])[:, :, me]          # [H, W, M]
        por_r = np.asarray(portion_logits[b])[:, :, mq]        # [H, W, M]
        bin_r = np.asarray(binary_mask_logits[b])[:, :, mq]    # [H, W, M]
        twin_b = np.ascontiguousarray(true_r.reshape(FULLPIX, M).T).astype(f8)
        bwin_b = np.ascontiguousarray(bin_r.reshape(FULLPIX, M).T).astype(f8)
        iel = np.asarray(is_electron_logit, dtype=f).reshape(B, Q)[b].reshape(32, 5)
        lab = np.zeros(Q, dtype=f)
        lab[mq] = 1.0
        lab = lab.reshape(32, 5)
        occ_b = np.asarray(occupancy_logits[b], dtype=f)
        occt_b = np.asarray(occupancy_true[b])
        xsel_b = np.take_along_axis(occ_b.reshape(FULLPIX, K),
                                    occt_b.reshape(FULLPIX, 1), axis=1)
        for h in range(2):
            sl = slice(h * HALF, (h + 1) * HALF)
            psl = slice(h * NPIX, (h + 1) * NPIX)
            smalls = np.zeros((P, NSM), dtype=f)
            smalls[0:M, 0:2] = np.asarray(incidence_points[b], dtype=f)[me]
            smalls[0:M, 2:4] = np.asarray(positions[b], dtype=f)[mq]
            smalls[0:M, 4:8] = np.asarray(chol[b], dtype=f).reshape(Q, 4)[mq]
            smalls[0:M, 8:15] = np.arange(WIN, dtype=f)[None, :]
            smalls[0:M, 15] = float(h * HALF)
            smalls[0:M, 16] = float(h * HALF + HALF - 1)
            smalls[0:M, 17] = np.arange(M, dtype=f) * FULLPIX
            smalls[0:32, 18:23] = iel
            smalls[0:32, 23:28] = lab
            in_maps.append(dict(
                por_sl=np.ascontiguousarray(por_r[sl]).reshape(NPIX, M).astype(f8),
                true_sl=np.ascontiguousarray(true_r[sl]).reshape(NPIX, M).astype(f8e4),
                occ_sl=np.ascontiguousarray(occ_b[sl]).reshape(P, J, K).astype(f8),
                xsel=np.ascontiguousarray(xsel_b[psl]).reshape(P, J).astype(bf),
                twin=twin_b, bwin=bwin_b,
                smalls=smalls, ident=ident,
            ))
    return in_maps


def combine(partials_list):
    s = np.stack([np.asarray(p, dtype=np.float64).reshape(8) for p in partials_list])
    # slots: 0=nll_sum 1=bce_sum 2=trace(C) 3=sum(C)=sum_true 4=occ_sum
    # 5=softplus_sum 6=matched(0.9*sp - x) sum
    class_loss = (NO_E * s[0::2, 5].sum() + s[0::2, 6].sum()) / (B * Q)
    nll_loss = s[0::2, 0].sum() / (B * M)
    bce_loss = s[:, 1].sum() / (B * M * WIN * WIN)
    occ_loss = s[:, 4].sum() / (B * H * W)
    dice = 0.0
    for b in range(B):
        num = 2.0 * (s[2 * b, 2] + s[2 * b + 1, 2])
        den = s[2 * b, 3] + s[2 * b + 1, 3] + H * W
        dice += 1.0 - (num + 1.0) / (den + 1.0)
    dice_loss = dice / B
    return np.float32(class_loss + bce_loss + dice_loss + nll_loss + occ_loss)


def kernel(**inputs):
    from concourse.bass_utils import run_bass_kernel_spmd
    nc = _get_nc()
    in_maps = make_in_maps(**{k: np.asarray(v) for k, v in inputs.items()})
    r = run_bass_kernel_spmd(nc, in_maps, list(range(8)))
    return combine([r.results[c]["partials"] for c in range(8)])
